# revision 3
# baseline (speedup 1.0000x reference)
"""EEG-GAT 3-layer network on 8 Trainium2 NeuronCores (Bass/Tile, single NEFF).

Sharding: data-parallel over nodes. Core k owns nodes [3750k, 3750(k+1)), padded
to 3840. Per layer: local matmul -> AllGather of the f32 node table
[h(256) | a_src-logit(4) | a_dst-logit(4) | pad] -> per-dst-block edge phase
(dma_gather of source rows + one-hot scatter matmuls in PSUM) -> epilogue
(softmax-divide, fused BN affine, ELU). Graph pooling via one-hot matmul; host
divides by graph sizes and assembles the [1500, 64] output.
"""
import sys
sys.path.insert(0, '/opt/trn_rl_repo')
import numpy as np
import ml_dtypes

N, FIN, H, C = 30000, 3000, 4, 64
HC = H * C                     # 256
G_GRAPHS = 1500
N_CORES = 8
NLOC = N // N_CORES            # 3750
NPAD = 3840                    # per-core padded nodes (30 blocks of 128)
NBLK = NPAD // 128             # 30
ROW = 320                      # table row (f32 elems); 1280 B, %256 ok
KP1 = 3072                     # padded FIN for layer-1 K tiles
NEG_SLOPE = 0.2
BN_EPS = 1e-5
SPAN = 256                     # graph-span tiles per core (2 x 128)

bf16 = ml_dtypes.bfloat16


# ----------------------------------------------------------------------------
# host-side preprocessing
# ----------------------------------------------------------------------------

def _w_ext(W, a_s, a_d, fin_pad):
    """[fin_pad, 264] = [W | W@blockdiag(a_src) | W@blockdiag(a_dst)] (f32)."""
    fin = W.shape[0]
    avec = np.zeros((HC, 2 * H), np.float32)
    for h in range(H):
        avec[h * C:(h + 1) * C, h] = a_s[h]
        avec[h * C:(h + 1) * C, H + h] = a_d[h]
    We = np.zeros((fin_pad, HC + 2 * H), np.float32)
    We[:fin, :HC] = W
    We[:fin, HC:] = W.astype(np.float64) @ avec.astype(np.float64)
    return We


def _affine(bias, g, b, m, v):
    s = (g / np.sqrt(v + BN_EPS)).astype(np.float32)
    t = ((bias - m) * s + b).astype(np.float32)
    return s, t


def _preprocess(inputs):
    x = np.asarray(inputs['x'], np.float32)
    ei = np.asarray(inputs['edge_index']).astype(np.int64)
    batch = np.asarray(inputs['batch']).astype(np.int64)

    src = np.concatenate([ei[0], np.arange(N, dtype=np.int64)])
    dst = np.concatenate([ei[1], np.arange(N, dtype=np.int64)])

    per_core = []
    n_chunks = 0
    for k in range(N_CORES):
        lo, hi = NLOC * k, NLOC * (k + 1)
        sel = (dst >= lo) & (dst < hi)
        s_k, d_k = src[sel], dst[sel] - lo
        order = np.argsort(d_k, kind='stable')
        s_k, d_k = s_k[order], d_k[order]
        blk = d_k // 128
        cnts = np.bincount(blk, minlength=NBLK)
        n_chunks = max(n_chunks, int(np.ceil(cnts.max() / 128)))
        per_core.append((s_k, d_k, blk, cnts))

    NCH = n_chunks
    NI = NCH * 128

    # graph spans
    g_starts = [int(batch[NLOC * k]) for k in range(N_CORES)]
    for k in range(N_CORES):
        span = int(batch[NLOC * (k + 1) - 1]) - g_starts[k] + 1
        assert span <= SPAN, f"graph span {span} > {SPAN}"

    ins = []
    for k in range(N_CORES):
        s_k, d_k, blk, cnts = per_core[k]
        idxm = np.zeros((NBLK, NI), np.int64)          # padded src (global, 3840-remap)
        dstf = np.zeros((NBLK, NI), np.float32)        # dst-in-block
        maskf = np.zeros((NBLK, NI), np.float32)
        off = 0
        for b in range(NBLK):
            cnt = int(cnts[b])
            sb = s_k[off:off + cnt]
            db = d_k[off:off + cnt] - b * 128
            off += cnt
            idxm[b, :cnt] = (sb // NLOC) * NPAD + (sb % NLOC)
            dstf[b, :cnt] = db.astype(np.float32)
            maskf[b, :cnt] = 1.0
        # wrap gather indices: idx j -> [j%16, j//16], replicate over 8 groups
        wrapped = idxm.reshape(NBLK, NI // 16, 16).transpose(0, 2, 1)
        idx_in = np.tile(wrapped, (1, 8, 1)).astype(np.int16)
        # [NBLK, 128, NCH] layouts: edge (c*128+p) at [p, c]
        dst_in = dstf.reshape(NBLK, NCH, 128).transpose(0, 2, 1).copy()
        mask_in = maskf.reshape(NBLK, NCH, 128).transpose(0, 2, 1).copy()

        padfix = np.zeros((NBLK, 128, 1), np.float32)
        batchg = np.full((NBLK, 128, 1), -1.0, np.float32)
        for b in range(NBLK):
            for p in range(128):
                node = b * 128 + p
                if node >= NLOC:
                    padfix[b, p, 0] = 1.0
                else:
                    batchg[b, p, 0] = float(batch[NLOC * k + node] - g_starts[k])

        xT = np.zeros((KP1, NPAD), bf16)
        xT[:FIN, :NLOC] = x[NLOC * k:NLOC * (k + 1)].T.astype(bf16)

        m = {'xT': xT, 'idxm': idx_in, 'dstf': dst_in, 'maskf': mask_in,
             'padfix': padfix, 'batchg': batchg}
        ins.append(m)

    # shared (same on all cores)
    W1e = _w_ext(np.asarray(inputs['W1'], np.float32), np.asarray(inputs['as1'], np.float32),
                 np.asarray(inputs['ad1'], np.float32), KP1).astype(bf16)
    W2e = _w_ext(np.asarray(inputs['W2'], np.float32), np.asarray(inputs['as2'], np.float32),
                 np.asarray(inputs['ad2'], np.float32), HC).astype(bf16)
    W3e = _w_ext(np.asarray(inputs['W3'], np.float32), np.asarray(inputs['as3'], np.float32),
                 np.asarray(inputs['ad3'], np.float32), HC).astype(bf16)
    s1, t1 = _affine(np.asarray(inputs['b1'], np.float32), np.asarray(inputs['bn1_g'], np.float32),
                     np.asarray(inputs['bn1_b'], np.float32), np.asarray(inputs['bn1_m'], np.float32),
                     np.asarray(inputs['bn1_v'], np.float32))
    s2, t2 = _affine(np.asarray(inputs['b2'], np.float32), np.asarray(inputs['bn2_g'], np.float32),
                     np.asarray(inputs['bn2_b'], np.float32), np.asarray(inputs['bn2_m'], np.float32),
                     np.asarray(inputs['bn2_v'], np.float32))
    s3, t3 = _affine(np.asarray(inputs['b3'], np.float32), np.asarray(inputs['bn3_g'], np.float32),
                     np.asarray(inputs['bn3_b'], np.float32), np.asarray(inputs['bn3_m'], np.float32),
                     np.asarray(inputs['bn3_v'], np.float32))
    iota256 = np.tile(np.arange(256, dtype=np.float32)[None, :], (128, 1))
    ident = np.eye(128, dtype=np.float32)
    st1 = np.stack([np.tile(s1, (128, 1)), np.tile(t1, (128, 1))])
    st2 = np.stack([np.tile(s2, (128, 1)), np.tile(t2, (128, 1))])
    st3 = np.stack([np.tile(s3, (128, 1)), np.tile(t3, (128, 1))])
    shared = {'W1e': W1e, 'W2e': W2e, 'W3e': W3e, 'st1': st1, 'st2': st2,
              'st3': st3, 'iota256': iota256, 'identf': ident}
    for m in ins:
        m.update(shared)

    counts = np.bincount(batch, minlength=G_GRAPHS).astype(np.float32)
    return ins, NCH, g_starts, counts


# ----------------------------------------------------------------------------
# device program
# ----------------------------------------------------------------------------

_PROG_CACHE = {}


def _build_program(NCH):
    import concourse.bass as bass
    import concourse.bacc as bacc
    import concourse.tile as tile
    import concourse.mybir as mybir
    from contextlib import ExitStack

    f32 = mybir.dt.float32
    b16 = mybir.dt.bfloat16
    NI = NCH * 128
    Alu = mybir.AluOpType
    Act = mybir.ActivationFunctionType

    nc = bacc.Bacc("TRN2", target_bir_lowering=False, debug=False,
                   num_devices=N_CORES, num_swdge_queues=2)

    # inputs
    xT = nc.dram_tensor("xT", [KP1, NPAD], b16, kind="ExternalInput")
    W1e = nc.dram_tensor("W1e", [KP1, 264], b16, kind="ExternalInput")
    W2e = nc.dram_tensor("W2e", [HC, 264], b16, kind="ExternalInput")
    W3e = nc.dram_tensor("W3e", [HC, 264], b16, kind="ExternalInput")
    idxm = nc.dram_tensor("idxm", [NBLK, 128, NI // 16], mybir.dt.int16, kind="ExternalInput")
    dstf = nc.dram_tensor("dstf", [NBLK, 128, NCH], f32, kind="ExternalInput")
    maskf = nc.dram_tensor("maskf", [NBLK, 128, NCH], f32, kind="ExternalInput")
    padfix = nc.dram_tensor("padfix", [NBLK, 128, 1], f32, kind="ExternalInput")
    batchg = nc.dram_tensor("batchg", [NBLK, 128, 1], f32, kind="ExternalInput")
    st1 = nc.dram_tensor("st1", [2, 128, 256], f32, kind="ExternalInput")
    st2 = nc.dram_tensor("st2", [2, 128, 256], f32, kind="ExternalInput")
    st3 = nc.dram_tensor("st3", [2, 128, 64], f32, kind="ExternalInput")
    iota256 = nc.dram_tensor("iota256", [128, 256], f32, kind="ExternalInput")
    identf = nc.dram_tensor("identf", [128, 128], f32, kind="ExternalInput")
    pout = nc.dram_tensor("pout", [SPAN, 64], f32, kind="ExternalOutput")

    # internals
    ag_in = [nc.dram_tensor(f"ag_in{l}", [NPAD, ROW], f32, kind="Internal")
             for l in range(3)]
    table = [nc.dram_tensor(f"table{l}", [N_CORES * NPAD, ROW], f32,
                            kind="Internal", addr_space="Shared") for l in range(3)]
    aux = [nc.dram_tensor(f"aux{l}", [NPAD, H], f32, kind="Internal")
           for l in range(3)]

    with ExitStack() as es:
        tc = es.enter_context(tile.TileContext(nc))
        cp = es.enter_context(tc.tile_pool(name="consts", bufs=1))
        mmp = es.enter_context(tc.tile_pool(name="mmps", bufs=2, space="PSUM"))
        stp = es.enter_context(tc.tile_pool(name="staging", bufs=3))
        ytp = es.enter_context(tc.tile_pool(name="yt", bufs=1))

        # constants
        identb = cp.tile([128, 128], b16, tag="identb")
        iot = cp.tile([128, 256], f32, tag="iota")
        nc.sync.dma_start(out=iot[:], in_=iota256[:])
        idf = cp.tile([128, 128], f32, tag="identf")
        nc.sync.dma_start(out=idf[:], in_=identf[:])
        nc.vector.tensor_copy(out=identb[:], in_=idf[:])
        stt = {}
        for l, st_ in ((1, st1), (2, st2), (3, st3)):
            w = 256 if l < 3 else 64
            for j, nm in ((0, 's'), (1, 't')):
                tl = cp.tile([128, w], f32, tag=f"st{l}{nm}")
                nc.sync.dma_start(out=tl[:], in_=st_[j])
                stt[(l, nm)] = tl

        yT = {}
        for ab in 'AB':
            for kk in range(2):
                yT[(ab, kk)] = ytp.tile([128, NPAD], b16, tag=f"yT{ab}{kk}",
                                        name=f"yT{ab}{kk}")

        def mm_phase(l):
            """local matmul -> staging -> ag_in[l-1] + aux[l-1] rows."""
            li = l - 1
            if l == 1:
                wp = es_l1[0]
                xp = es_l1[1]
                NKT = KP1 // 128
                w1t = []
                for k in range(NKT):
                    w = wp.tile([128, 264], b16, tag=f"w1_{k}")
                    nc.sync.dma_start(out=w[:], in_=W1e[k * 128:(k + 1) * 128, :])
                    w1t.append(w)
                MW = 3  # m-tiles per window
                for wdw in range(NBLK // MW):
                    xw = []
                    for k in range(NKT):
                        xt = xp.tile([128, MW * 128], b16, tag=f"xw{k}")
                        nc.sync.dma_start(
                            out=xt[:],
                            in_=xT[k * 128:(k + 1) * 128,
                                   wdw * MW * 128:(wdw + 1) * MW * 128])
                        xw.append(xt)
                    for mi in range(MW):
                        m = wdw * MW + mi
                        ps = mmp.tile([128, 264], f32, space="PSUM", tag="mmps")
                        for k in range(NKT):
                            nc.tensor.matmul(out=ps[:], lhsT=xw[k][:, mi * 128:(mi + 1) * 128],
                                             rhs=w1t[k][:], start=(k == 0), stop=(k == NKT - 1))
                        _mm_store(li, m, ps)
            else:
                wts = w23[l]
                src_ab = 'A' if l == 2 else 'B'
                for m in range(NBLK):
                    ps = mmp.tile([128, 264], f32, space="PSUM", tag="mmps")
                    for kk in range(2):
                        nc.tensor.matmul(out=ps[:], lhsT=yT[(src_ab, kk)][:, m * 128:(m + 1) * 128],
                                         rhs=wts[kk][:], start=(kk == 0), stop=(kk == 1))
                    _mm_store(li, m, ps)

        def _mm_store(li, m, ps):
            stg = stp.tile([128, 264], f32, tag="stg")
            nc.vector.tensor_copy(out=stg[:], in_=ps[:])
            nc.sync.dma_start(out=ag_in[li].ap()[m * 128:(m + 1) * 128, 0:264], in_=stg[:])
            nc.sync.dma_start(out=aux[li].ap()[m * 128:(m + 1) * 128, :], in_=stg[:, 260:264])

        def edge_phase(l, pools):
            li = l - 1
            gp, op, otp, otsp, gsp, ep, pp_ot, pp_wps, pp_out, pp_pg = pools
            if l == 3:
                pacc = [ep.tile([128, 64], f32, tag=f"pacc{gg}", name=f"pacc{gg}")
                        for gg in range(2)]
                for gg in range(2):
                    nc.vector.memset(pacc[gg][:], 0.0)
            for b in range(NBLK):
                idx_s = ep.tile([128, NI // 16], mybir.dt.int16, tag="idx")
                nc.sync.dma_start(out=idx_s[:], in_=idxm[b])
                dst_s = ep.tile([128, NCH], f32, tag="dst")
                nc.sync.dma_start(out=dst_s[:], in_=dstf[b])
                mask_s = ep.tile([128, NCH], f32, tag="mask")
                nc.sync.dma_start(out=mask_s[:], in_=maskf[b])
                pfx_s = ep.tile([128, 1], f32, tag="pfx")
                nc.sync.dma_start(out=pfx_s[:], in_=padfix[b])
                adB = ep.tile([128, H], f32, tag="adB")
                nc.sync.dma_start(out=adB[:], in_=aux[li].ap()[b * 128:(b + 1) * 128, :])
                adBb = ep.tile([128, H], b16, tag="adBb")
                nc.vector.tensor_copy(out=adBb[:], in_=adB[:])

                G = gp.tile([128, NCH, ROW], f32, tag="G")
                nc.gpsimd.dma_gather(out_ap=G[:], in_ap=table[li].ap()[:],
                                     idxs_ap=idx_s[:], num_idxs=NI, num_idxs_reg=NI,
                                     elem_size=ROW, single_packet=False,
                                     queue_num=b % 2)

                onehot = op.tile([128, NCH, 128], b16, tag="onehot")
                nc.vector.tensor_tensor(
                    out=onehot[:],
                    in0=iot[:, None, 0:128].to_broadcast([128, NCH, 128]),
                    in1=dst_s[:, :, None].to_broadcast([128, NCH, 128]),
                    op=Alu.is_equal)

                wps = pp_wps.tile([128, NCH * H], f32, space="PSUM", tag="wps")
                for c in range(NCH):
                    oT_ps = pp_ot.tile([128, 128], b16, space="PSUM", tag="ot")
                    nc.tensor.transpose(out=oT_ps[:], in_=onehot[:, c, :], identity=identb[:])
                    oT = otsp.tile([128, 128], b16, tag="oTs")
                    nc.vector.tensor_copy(out=oT[:], in_=oT_ps[:])
                    nc.tensor.matmul(out=wps[:, c * H:(c + 1) * H], lhsT=oT[:],
                                     rhs=adBb[:], start=True, stop=True)

                as_sl = G[:, :, 256:260]
                nc.vector.tensor_tensor(out=as_sl, in0=as_sl,
                                        in1=wps[:].rearrange("p (c h) -> p c h", h=H),
                                        op=Alu.add)
                lk = ep.tile([128, NCH, H], f32, tag="lk")
                nc.vector.tensor_scalar_mul(out=lk[:], in0=as_sl, scalar1=NEG_SLOPE)
                nc.vector.tensor_tensor(out=as_sl, in0=as_sl, in1=lk[:], op=Alu.max)
                nc.scalar.activation(out=as_sl, in_=as_sl, func=Act.Exp)
                nc.vector.tensor_tensor(out=as_sl, in0=as_sl,
                                        in1=mask_s[:, :, None].to_broadcast([128, NCH, H]),
                                        op=Alu.mult)

                Gs = gsp.tile([128, NCH, 260], b16, tag="Gs")
                nc.vector.tensor_tensor(
                    out=Gs[:, :, 0:256].rearrange("p c (h x) -> p c h x", h=H),
                    in0=G[:, :, 0:256].rearrange("p c (h x) -> p c h x", h=H),
                    in1=G[:, :, 256:260][:, :, :, None].to_broadcast([128, NCH, H, C]),
                    op=Alu.mult)
                nc.vector.tensor_copy(out=Gs[:, :, 256:260], in_=G[:, :, 256:260])

                out_ps = pp_out.tile([128, 260], f32, space="PSUM", tag="outps")
                for c in range(NCH):
                    nc.tensor.matmul(out=out_ps[:], lhsT=onehot[:, c, :], rhs=Gs[:, c, :],
                                     start=(c == 0), stop=(c == NCH - 1))

                # epilogue
                dsafe = ep.tile([128, H], f32, tag="dsafe")
                nc.vector.tensor_tensor(out=dsafe[:], in0=out_ps[:, 256:260],
                                        in1=pfx_s[:].to_broadcast([128, H]), op=Alu.add)
                rec = ep.tile([128, H], f32, tag="rec")
                nc.vector.reciprocal(out=rec[:], in_=dsafe[:])

                if l < 3:
                    y = ep.tile([128, 256], f32, tag="y")
                    nc.vector.tensor_tensor(
                        out=y[:].rearrange("p (h x) -> p h x", h=H),
                        in0=out_ps[:, 0:256].rearrange("p (h x) -> p h x", h=H),
                        in1=rec[:, :, None].to_broadcast([128, H, C]), op=Alu.mult)
                    w_ = 256
                else:
                    rec25 = ep.tile([128, H], f32, tag="rec25")
                    nc.vector.tensor_scalar_mul(out=rec25[:], in0=rec[:], scalar1=0.25)
                    tmp = ep.tile([128, 256], f32, tag="tmp3")
                    nc.vector.tensor_tensor(
                        out=tmp[:].rearrange("p (h x) -> p h x", h=H),
                        in0=out_ps[:, 0:256].rearrange("p (h x) -> p h x", h=H),
                        in1=rec25[:, :, None].to_broadcast([128, H, C]), op=Alu.mult)
                    y = ep.tile([128, 64], f32, tag="y3")
                    nc.vector.tensor_reduce(
                        out=y[:],
                        in_=tmp[:].rearrange("p (h x) -> p x h", h=H),
                        axis=mybir.AxisListType.X, op=Alu.add)
                    w_ = 64

                # fused BN affine + ELU
                nc.vector.tensor_tensor(out=y[:], in0=y[:], in1=stt[(l, 's')][:, :w_], op=Alu.mult)
                nc.vector.tensor_tensor(out=y[:], in0=y[:], in1=stt[(l, 't')][:, :w_], op=Alu.add)
                mn = ep.tile([128, w_], f32, tag=f"mn{w_}")
                nc.vector.tensor_scalar_min(out=mn[:], in0=y[:], scalar1=0.0)
                nc.scalar.activation(out=mn[:], in_=mn[:], func=Act.Exp)
                nc.vector.tensor_scalar_max(out=y[:], in0=y[:], scalar1=0.0)
                nc.vector.tensor_tensor(out=y[:], in0=y[:], in1=mn[:], op=Alu.add)

                if l < 3:
                    y4b = ep.tile([128, 256], b16, tag="y4b")
                    nc.vector.tensor_scalar_add(out=y4b[:], in0=y[:], scalar1=-1.0)
                    dst_ab = 'A' if l == 1 else 'B'
                    for kk in range(2):
                        tp = pp_ot.tile([128, 128], b16, space="PSUM", tag="ot")
                        nc.tensor.transpose(out=tp[:], in_=y4b[:, kk * 128:(kk + 1) * 128],
                                            identity=identb[:])
                        nc.vector.tensor_copy(out=yT[(dst_ab, kk)][:, b * 128:(b + 1) * 128],
                                              in_=tp[:])
                else:
                    yp = ep.tile([128, 64], f32, tag="yp")
                    nc.vector.tensor_scalar_add(out=yp[:], in0=y[:], scalar1=-1.0)
                    bg_s = ep.tile([128, 1], f32, tag="bg")
                    nc.sync.dma_start(out=bg_s[:], in_=batchg[b])
                    og = ep.tile([128, 256], f32, tag="og")
                    nc.vector.tensor_tensor(out=og[:],
                                            in0=bg_s[:].to_broadcast([128, 256]),
                                            in1=iot[:], op=Alu.is_equal)
                    for gg in range(2):
                        pg = pp_pg.tile([128, 64], f32, space="PSUM", tag="pg")
                        nc.tensor.matmul(out=pg[:], lhsT=og[:, gg * 128:(gg + 1) * 128],
                                         rhs=yp[:], start=True, stop=True)
                        nc.vector.tensor_tensor(out=pacc[gg][:], in0=pacc[gg][:],
                                                in1=pg[:], op=Alu.add)
            if l == 3:
                for gg in range(2):
                    nc.sync.dma_start(out=pout.ap()[gg * 128:(gg + 1) * 128, :],
                                      in_=pacc[gg][:])

        def do_ag(l):
            li = l - 1
            nc.gpsimd.collective_compute(
                "AllGather", Alu.bypass,
                replica_groups=[list(range(N_CORES))],
                ins=[ag_in[li].ap().opt()], outs=[table[li].ap().opt()])

        # ---- layer 1 ----
        with ExitStack() as es1:
            es_l1 = (es1.enter_context(tc.tile_pool(name="w1", bufs=1)),
                     es1.enter_context(tc.tile_pool(name="xw", bufs=2)))
            mm_phase(1)
        # edge pools (allocated after w1/xw release)
        gp = es.enter_context(tc.tile_pool(name="G", bufs=2))
        op = es.enter_context(tc.tile_pool(name="onehot", bufs=2))
        otp = None
        otsp = es.enter_context(tc.tile_pool(name="oTs", bufs=3))
        gsp = es.enter_context(tc.tile_pool(name="Gs", bufs=2))
        ep = es.enter_context(tc.tile_pool(name="edge", bufs=3))
        pp_ot = es.enter_context(tc.tile_pool(name="pot", bufs=2, space="PSUM"))
        pp_wps = es.enter_context(tc.tile_pool(name="pwps", bufs=1, space="PSUM"))
        pp_out = es.enter_context(tc.tile_pool(name="pout", bufs=2, space="PSUM"))
        pp_pg = es.enter_context(tc.tile_pool(name="ppg", bufs=1, space="PSUM"))
        pools = (gp, op, otp, otsp, gsp, ep, pp_ot, pp_wps, pp_out, pp_pg)
        w23 = {}
        w23p = es.enter_context(tc.tile_pool(name="w23", bufs=1))
        for l, Wt in ((2, W2e), (3, W3e)):
            w23[l] = []
            for kk in range(2):
                w = w23p.tile([128, 264], b16, tag=f"w{l}_{kk}")
                nc.sync.dma_start(out=w[:], in_=Wt[kk * 128:(kk + 1) * 128, :])
                w23[l].append(w)

        do_ag(1)
        edge_phase(1, pools)
        mm_phase(2)
        do_ag(2)
        edge_phase(2, pools)
        mm_phase(3)
        do_ag(3)
        edge_phase(3, pools)

    nc.compile()
    return nc


# ----------------------------------------------------------------------------
# entry point
# ----------------------------------------------------------------------------

def kernel(**inputs):
    from concourse.bass_utils import run_bass_kernel_spmd

    in_maps, NCH, g_starts, counts = _preprocess(inputs)
    if NCH not in _PROG_CACHE:
        _PROG_CACHE[NCH] = _build_program(NCH)
    nc = _PROG_CACHE[NCH]

    res = run_bass_kernel_spmd(nc, in_maps, core_ids=list(range(N_CORES)))

    full = np.zeros((G_GRAPHS + SPAN, 64), np.float64)
    for k in range(N_CORES):
        full[g_starts[k]:g_starts[k] + SPAN] += res.results[k]['pout'].astype(np.float64)
    out = full[:G_GRAPHS] / np.maximum(counts, 1.0)[:, None]
    return out.astype(np.float32)


# revision 15
# speedup vs baseline: 5.0332x; 5.0332x over previous
"""EEG-GAT 3-layer network on 8 Trainium2 NeuronCores (Bass/Tile, single NEFF).

Sharding: data-parallel over nodes. Core k owns nodes [3750k, 3750(k+1)), padded
to 3840. Per layer: local matmul -> AllGather of the bf16 node table
[h(256) | a_src-logit(4) | a_dst-logit(4) | pad] -> per-dst-block edge phase
(dma_gather of source rows + one-hot scatter matmuls in PSUM) -> epilogue
(softmax-divide, fused BN affine, ELU). Graph pooling via one-hot matmul; host
divides by graph sizes and assembles the [1500, 64] output.
"""
import sys
sys.path.insert(0, '/opt/trn_rl_repo')
import numpy as np
import ml_dtypes

N, FIN, H, C = 30000, 3000, 4, 64
HC = H * C                     # 256
G_GRAPHS = 1500
N_CORES = 8
NLOC = N // N_CORES            # 3750
NPAD = 3840                    # per-core padded nodes (30 blocks of 128)
NBLK = NPAD // 128             # 30
ROW = 384                      # table row (bf16 elems); 768 B, %256 ok
KP1 = 3072                     # padded FIN for layer-1 K tiles
NEG_SLOPE = 0.2
BN_EPS = 1e-5
SPAN = 256                     # graph-span tiles per core (2 x 128)

bf16 = ml_dtypes.bfloat16


# ----------------------------------------------------------------------------
# host-side preprocessing
# ----------------------------------------------------------------------------

def _w_ext(W, a_s, a_d, fin_pad):
    """[fin_pad, 264] = [W | W@blockdiag(a_src) | W@blockdiag(a_dst)] (f32)."""
    fin = W.shape[0]
    avec = np.zeros((HC, 2 * H), np.float32)
    for h in range(H):
        avec[h * C:(h + 1) * C, h] = a_s[h]
        avec[h * C:(h + 1) * C, H + h] = a_d[h]
    We = np.zeros((fin_pad, HC + 2 * H), np.float32)
    We[:fin, :HC] = W
    We[:fin, HC:] = W.astype(np.float64) @ avec.astype(np.float64)
    return We


def _affine(bias, g, b, m, v):
    s = (g / np.sqrt(v + BN_EPS)).astype(np.float32)
    t = ((bias - m) * s + b).astype(np.float32)
    return s, t


def _preprocess(inputs):
    x = np.asarray(inputs['x'], np.float32)
    ei = np.asarray(inputs['edge_index']).astype(np.int64)
    batch = np.asarray(inputs['batch']).astype(np.int64)

    src = np.concatenate([ei[0], np.arange(N, dtype=np.int64)])
    dst = np.concatenate([ei[1], np.arange(N, dtype=np.int64)])

    per_core = []
    n_chunks = 0
    for k in range(N_CORES):
        lo, hi = NLOC * k, NLOC * (k + 1)
        sel = (dst >= lo) & (dst < hi)
        s_k, d_k = src[sel], dst[sel] - lo
        order = np.argsort(d_k, kind='stable')
        s_k, d_k = s_k[order], d_k[order]
        blk = d_k // 128
        cnts = np.bincount(blk, minlength=NBLK)
        n_chunks = max(n_chunks, int(np.ceil(cnts.max() / 128)))
        per_core.append((s_k, d_k, blk, cnts))

    NCH = n_chunks
    NI = NCH * 128

    g_starts = [int(batch[NLOC * k]) for k in range(N_CORES)]
    for k in range(N_CORES):
        span = int(batch[NLOC * (k + 1) - 1]) - g_starts[k] + 1
        assert span <= SPAN, f"graph span {span} > {SPAN}"

    eye128 = np.eye(128, dtype=np.float32)
    ins = []
    for k in range(N_CORES):
        s_k, d_k, blk, cnts = per_core[k]
        idxm = np.zeros((NBLK, NI), np.int64)
        dstf = np.zeros((NBLK, NI), np.float32)
        maskf = np.zeros((NBLK, NI), np.float32)
        off = 0
        for b in range(NBLK):
            cnt = int(cnts[b])
            sb = s_k[off:off + cnt]
            db = d_k[off:off + cnt] - b * 128
            off += cnt
            gid = (sb // NLOC) * NPAD + (sb % NLOC)
            # sort this block's edges by gather index for HBM locality
            o2 = np.argsort(gid, kind='stable')
            idxm[b, :cnt] = gid[o2]
            dstf[b, :cnt] = db[o2].astype(np.float32)
            maskf[b, :cnt] = 1.0
        wrapped = idxm.reshape(NBLK, NI // 16, 16).transpose(0, 2, 1)
        idx_in = np.tile(wrapped, (1, 8, 1)).astype(np.int16)
        dst_in = dstf.reshape(NBLK, NCH, 128).transpose(0, 2, 1).copy()
        mask_in = maskf.reshape(NBLK, NCH, 128).transpose(0, 2, 1).copy()

        # transposed one-hot per block: [128 nodes, NI edges] (edge order)
        ohT = np.empty((NBLK, 128, NI), bf16)
        for b in range(NBLK):
            ohT[b] = eye128[:, dstf[b].astype(np.int64)].astype(bf16)

        # block meta: [128, 2*NCH+2] = [dst | mask | padfix | batchg]
        meta = np.zeros((NBLK, 128, 2 * NCH + 2), np.float32)
        meta[:, :, 0:NCH] = dst_in
        meta[:, :, NCH:2 * NCH] = mask_in
        nodes = np.arange(NPAD).reshape(NBLK, 128)
        fake = nodes >= NLOC
        meta[:, :, 2 * NCH] = fake.astype(np.float32)
        bg = np.full((NBLK, 128), -1.0, np.float32)
        bg[~fake] = (batch[NLOC * k:NLOC * (k + 1)] - g_starts[k]).astype(np.float32)
        meta[:, :, 2 * NCH + 1] = bg

        xT = np.zeros((KP1, NPAD), bf16)
        xT[:FIN, :NLOC] = x[NLOC * k:NLOC * (k + 1)].T.astype(bf16)

        ins.append({'xT': xT, 'idxm': idx_in, 'meta': meta, 'ohT': ohT})

    W1e = _w_ext(np.asarray(inputs['W1'], np.float32), np.asarray(inputs['as1'], np.float32),
                 np.asarray(inputs['ad1'], np.float32), KP1).astype(bf16)
    W2e = _w_ext(np.asarray(inputs['W2'], np.float32), np.asarray(inputs['as2'], np.float32),
                 np.asarray(inputs['ad2'], np.float32), HC).astype(bf16)
    W3e = _w_ext(np.asarray(inputs['W3'], np.float32), np.asarray(inputs['as3'], np.float32),
                 np.asarray(inputs['ad3'], np.float32), HC).astype(bf16)
    s1, t1 = _affine(np.asarray(inputs['b1'], np.float32), np.asarray(inputs['bn1_g'], np.float32),
                     np.asarray(inputs['bn1_b'], np.float32), np.asarray(inputs['bn1_m'], np.float32),
                     np.asarray(inputs['bn1_v'], np.float32))
    s2, t2 = _affine(np.asarray(inputs['b2'], np.float32), np.asarray(inputs['bn2_g'], np.float32),
                     np.asarray(inputs['bn2_b'], np.float32), np.asarray(inputs['bn2_m'], np.float32),
                     np.asarray(inputs['bn2_v'], np.float32))
    s3, t3 = _affine(np.asarray(inputs['b3'], np.float32), np.asarray(inputs['bn3_g'], np.float32),
                     np.asarray(inputs['bn3_b'], np.float32), np.asarray(inputs['bn3_m'], np.float32),
                     np.asarray(inputs['bn3_v'], np.float32))
    iota256 = np.tile(np.arange(256, dtype=np.float32)[None, :], (128, 1))
    ident = np.eye(128, dtype=np.float32)
    st1 = np.stack([np.tile(s1, (128, 1)), np.tile(t1, (128, 1))])
    st2 = np.stack([np.tile(s2, (128, 1)), np.tile(t2, (128, 1))])
    st3 = np.stack([np.tile(s3, (128, 1)), np.tile(t3, (128, 1))])
    shared = {'W1e': W1e, 'W2e': W2e, 'W3e': W3e, 'st1': st1, 'st2': st2,
              'st3': st3, 'iota256': iota256, 'identf': ident}
    for m in ins:
        m.update(shared)

    counts = np.bincount(batch, minlength=G_GRAPHS).astype(np.float32)
    return ins, NCH, g_starts, counts


# ----------------------------------------------------------------------------
# device program
# ----------------------------------------------------------------------------

_PROG_CACHE = {}
KNOBS = {'G': 4, 'po': 3, 'wps': 1, 'oh': 2, 'gs': 2, 'ep': 3, 'mm': 2, 'pot': 2}


def _build_program(NCH, ablate=()):
    import concourse.bass as bass
    import concourse.bacc as bacc
    import concourse.tile as tile
    import concourse.mybir as mybir
    from contextlib import ExitStack

    f32 = mybir.dt.float32
    b16 = mybir.dt.bfloat16
    NI = NCH * 128
    Alu = mybir.AluOpType
    Act = mybir.ActivationFunctionType

    from concourse.hw_specs import TRN2Spec
    if not globals().get('_NO_PATCH', False):
        TRN2Spec.SWDGE_NS_PER_DESCRIPTOR = 7.7
        TRN2Spec.DMA_MIN_TRANSFER_TIME = 60
    else:
        TRN2Spec.SWDGE_NS_PER_DESCRIPTOR = 0.34
        TRN2Spec.DMA_MIN_TRANSFER_TIME = 7
    nc = bacc.Bacc("TRN2", target_bir_lowering=False, debug=False,
                   num_devices=N_CORES, num_swdge_queues=2)

    # inputs
    xT = nc.dram_tensor("xT", [KP1, NPAD], b16, kind="ExternalInput")
    W1e = nc.dram_tensor("W1e", [KP1, 264], b16, kind="ExternalInput")
    W2e = nc.dram_tensor("W2e", [HC, 264], b16, kind="ExternalInput")
    W3e = nc.dram_tensor("W3e", [HC, 264], b16, kind="ExternalInput")
    idxm = nc.dram_tensor("idxm", [NBLK, 128, NI // 16], mybir.dt.int16, kind="ExternalInput")
    meta = nc.dram_tensor("meta", [NBLK, 128, 2 * NCH + 2], f32, kind="ExternalInput")
    ohTd = nc.dram_tensor("ohT", [NBLK, 128, NI], b16, kind="ExternalInput")
    st1 = nc.dram_tensor("st1", [2, 128, 256], f32, kind="ExternalInput")
    st2 = nc.dram_tensor("st2", [2, 128, 256], f32, kind="ExternalInput")
    st3 = nc.dram_tensor("st3", [2, 128, 64], f32, kind="ExternalInput")
    iota256 = nc.dram_tensor("iota256", [128, 256], f32, kind="ExternalInput")
    identf = nc.dram_tensor("identf", [128, 128], f32, kind="ExternalInput")
    pout = nc.dram_tensor("pout", [SPAN, 64], f32, kind="ExternalOutput")

    # internals
    ag_in = [nc.dram_tensor(f"ag_in{l}", [NPAD, ROW], b16, kind="Internal")
             for l in range(3)]
    table = [nc.dram_tensor(f"table{l}", [N_CORES * NPAD, ROW], b16,
                            kind="Internal", addr_space="Shared") for l in range(3)]
    aux = [nc.dram_tensor(f"aux{l}", [NPAD, H], b16, kind="Internal")
           for l in range(3)]

    with ExitStack() as es:
        tc = es.enter_context(tile.TileContext(nc))
        cp = es.enter_context(tc.tile_pool(name="consts", bufs=1))
        mmp = es.enter_context(tc.tile_pool(name="mmps", bufs=KNOBS["mm"], space="PSUM"))
        stp = es.enter_context(tc.tile_pool(name="staging", bufs=3))
        ytp = es.enter_context(tc.tile_pool(name="yt", bufs=1))

        identb = cp.tile([128, 128], b16, tag="identb")
        iot = cp.tile([128, 256], f32, tag="iota")
        nc.sync.dma_start(out=iot[:], in_=iota256[:])
        idf = cp.tile([128, 128], f32, tag="identf")
        nc.sync.dma_start(out=idf[:], in_=identf[:])
        nc.vector.tensor_copy(out=identb[:], in_=idf[:])
        stt = {}
        for l, st_ in ((1, st1), (2, st2), (3, st3)):
            w = 256 if l < 3 else 64
            for j, nm in ((0, 's'), (1, 't')):
                tl = cp.tile([128, w], f32, tag=f"st{l}{nm}")
                nc.sync.dma_start(out=tl[:], in_=st_[j])
                stt[(l, nm)] = tl

        yT = {}
        for ab in 'AB':
            for kk in range(2):
                yT[(ab, kk)] = ytp.tile([128, NPAD], b16, tag=f"yT{ab}{kk}",
                                        name=f"yT{ab}{kk}")

        def _mm_store(li, m, ps):
            stg = stp.tile([128, 264], b16, tag="stg")
            nc.vector.tensor_copy(out=stg[:], in_=ps[:])
            nc.sync.dma_start(out=ag_in[li].ap()[m * 128:(m + 1) * 128, 0:264], in_=stg[:])
            nc.sync.dma_start(out=aux[li].ap()[m * 128:(m + 1) * 128, :], in_=stg[:, 260:264])

        def mm_phase(l):
            if 'mm' in ablate:
                return
            li = l - 1
            if l == 1:
                wp, xp = es_l1
                NKT = KP1 // 128
                w1t = []
                for k in range(NKT):
                    w = wp.tile([128, 264], b16, tag=f"w1_{k}")
                    nc.sync.dma_start(out=w[:], in_=W1e[k * 128:(k + 1) * 128, :])
                    w1t.append(w)
                MW = 3
                for wdw in range(NBLK // MW):
                    xw = []
                    for k in range(NKT):
                        xt = xp.tile([128, MW * 128], b16, tag=f"xw{k}")
                        nc.sync.dma_start(
                            out=xt[:],
                            in_=xT[k * 128:(k + 1) * 128,
                                   wdw * MW * 128:(wdw + 1) * MW * 128])
                        xw.append(xt)
                    for mi in range(MW):
                        m = wdw * MW + mi
                        ps = mmp.tile([128, 264], f32, space="PSUM", tag="acc")
                        for k in range(NKT):
                            nc.tensor.matmul(out=ps[:], lhsT=xw[k][:, mi * 128:(mi + 1) * 128],
                                             rhs=w1t[k][:], start=(k == 0), stop=(k == NKT - 1))
                        _mm_store(li, m, ps)
            else:
                wts = w23[l]
                src_ab = 'A' if l == 2 else 'B'
                for m in range(NBLK):
                    ps = mmp.tile([128, 264], f32, space="PSUM", tag="acc")
                    for kk in range(2):
                        nc.tensor.matmul(out=ps[:], lhsT=yT[(src_ab, kk)][:, m * 128:(m + 1) * 128],
                                         rhs=wts[kk][:], start=(kk == 0), stop=(kk == 1))
                    _mm_store(li, m, ps)

        def edge_phase(l, pools):
            li = l - 1
            gp, op, ohp, gsp, ep, pp_ot, pp_wps, pp_out, pp_pg = pools
            if l == 3:
                pacc = [ep.tile([128, 64], f32, tag=f"pacc{gg}", name=f"pacc{gg}")
                        for gg in range(2)]
                for gg in range(2):
                    nc.vector.memset(pacc[gg][:], 0.0)
            for b in range(NBLK):
                idx_s = ep.tile([128, NI // 16], mybir.dt.int16, tag="idx")
                nc.sync.dma_start(out=idx_s[:], in_=idxm[b])
                G = gp.tile([128, NCH, ROW], b16, tag="G")
                if 'gather' not in ablate:
                    nc.gpsimd.dma_gather(out_ap=G[:], in_ap=table[li].ap()[:],
                                         idxs_ap=idx_s[:], num_idxs=NI, num_idxs_reg=NI,
                                         elem_size=ROW, single_packet=False,
                                         queue_num=b % 2)
                else:
                    nc.vector.memset(G[:, 0, :], 1.0)
                meta_s = ep.tile([128, 2 * NCH + 2], f32, tag="meta")
                nc.sync.dma_start(out=meta_s[:], in_=meta[b])
                dst_s = meta_s[:, 0:NCH]
                mask_s = meta_s[:, NCH:2 * NCH]
                pfx_s = meta_s[:, 2 * NCH:2 * NCH + 1]
                bg_s = meta_s[:, 2 * NCH + 1:2 * NCH + 2]
                adBb = ep.tile([128, H], b16, tag="adBb")
                nc.sync.dma_start(out=adBb[:], in_=aux[li].ap()[b * 128:(b + 1) * 128, :])
                ohT_s = ohp.tile([128, NCH, 128], b16, tag="ohT")
                nc.sync.dma_start(out=ohT_s[:],
                                  in_=ohTd[b].rearrange("p (c e) -> p c e", e=128))

                onehot = op.tile([128, NCH, 128], b16, tag="onehot")
                nc.vector.tensor_tensor(
                    out=onehot[:],
                    in0=iot[:, None, 0:128].to_broadcast([128, NCH, 128]),
                    in1=dst_s[:, :, None].to_broadcast([128, NCH, 128]),
                    op=Alu.is_equal)

                # adE = onehotT.T @ adB per chunk; e = as + adE (f32)
                wt = ep.tile([128, NCH, H], f32, tag="wt")
                if 'ade' not in ablate:
                    wps = pp_wps.tile([128, NCH * H], f32, space="PSUM", tag="wps")
                    for c in range(NCH):
                        nc.tensor.matmul(out=wps[:, c * H:(c + 1) * H], lhsT=ohT_s[:, c, :],
                                         rhs=adBb[:], start=True, stop=True)
                    nc.vector.tensor_tensor(out=wt[:], in0=G[:, :, 256:260],
                                            in1=wps[:].rearrange("p (c h) -> p c h", h=H),
                                            op=Alu.add)
                else:
                    nc.vector.tensor_copy(out=wt[:], in_=G[:, :, 256:260])

                # w = exp(max(e, 0.2e)) * mask
                lk = ep.tile([128, NCH, H], f32, tag="lk")
                nc.vector.tensor_scalar_mul(out=lk[:], in0=wt[:], scalar1=NEG_SLOPE)
                nc.vector.tensor_tensor(out=wt[:], in0=wt[:], in1=lk[:], op=Alu.max)
                nc.scalar.activation(out=wt[:], in_=wt[:], func=Act.Exp)
                nc.vector.tensor_tensor(out=wt[:], in0=wt[:],
                                        in1=mask_s[:, :, None].to_broadcast([128, NCH, H]),
                                        op=Alu.mult)

                Gs = gsp.tile([128, NCH, 260], b16, tag="Gs")
                nc.vector.tensor_tensor(
                    out=Gs[:, :, 0:256].rearrange("p c (h x) -> p c h x", h=H),
                    in0=G[:, :, 0:256].rearrange("p c (h x) -> p c h x", h=H),
                    in1=wt[:, :, :, None].to_broadcast([128, NCH, H, C]),
                    op=Alu.mult)
                nc.vector.tensor_copy(out=Gs[:, :, 256:260], in_=wt[:])

                out_ps = pp_out.tile([128, 260], f32, space="PSUM", tag="outps")
                if 'scatter' not in ablate:
                    for c in range(NCH):
                        nc.tensor.matmul(out=out_ps[:], lhsT=onehot[:, c, :],
                                         rhs=Gs[:, c, :],
                                         start=(c == 0), stop=(c == NCH - 1))
                else:
                    nc.tensor.matmul(out=out_ps[:], lhsT=onehot[:, 0, :],
                                     rhs=Gs[:, 0, :], start=True, stop=True)

                # epilogue
                if 'epi' in ablate:
                    continue
                dsafe = ep.tile([128, H], f32, tag="dsafe")
                nc.vector.tensor_tensor(out=dsafe[:], in0=out_ps[:, 256:260],
                                        in1=pfx_s.to_broadcast([128, H]), op=Alu.add)
                rec = ep.tile([128, H], f32, tag="rec")
                nc.vector.reciprocal(out=rec[:], in_=dsafe[:])

                if l < 3:
                    y = ep.tile([128, 256], f32, tag="y")
                    nc.vector.tensor_tensor(
                        out=y[:].rearrange("p (h x) -> p h x", h=H),
                        in0=out_ps[:, 0:256].rearrange("p (h x) -> p h x", h=H),
                        in1=rec[:, :, None].to_broadcast([128, H, C]), op=Alu.mult)
                    w_ = 256
                else:
                    rec25 = ep.tile([128, H], f32, tag="rec25")
                    nc.vector.tensor_scalar_mul(out=rec25[:], in0=rec[:], scalar1=0.25)
                    tmp = ep.tile([128, 256], f32, tag="tmp3")
                    nc.vector.tensor_tensor(
                        out=tmp[:].rearrange("p (h x) -> p h x", h=H),
                        in0=out_ps[:, 0:256].rearrange("p (h x) -> p h x", h=H),
                        in1=rec25[:, :, None].to_broadcast([128, H, C]), op=Alu.mult)
                    y = ep.tile([128, 64], f32, tag="y3")
                    nc.vector.tensor_reduce(
                        out=y[:],
                        in_=tmp[:].rearrange("p (h x) -> p x h", h=H),
                        axis=mybir.AxisListType.X, op=Alu.add)
                    w_ = 64

                nc.vector.tensor_tensor(out=y[:], in0=y[:], in1=stt[(l, 's')][:, :w_], op=Alu.mult)
                nc.vector.tensor_tensor(out=y[:], in0=y[:], in1=stt[(l, 't')][:, :w_], op=Alu.add)
                mn = ep.tile([128, w_], f32, tag=f"mn{w_}")
                nc.vector.tensor_scalar_min(out=mn[:], in0=y[:], scalar1=0.0)
                nc.scalar.activation(out=mn[:], in_=mn[:], func=Act.Exp)
                nc.vector.tensor_scalar_max(out=y[:], in0=y[:], scalar1=0.0)
                nc.vector.tensor_tensor(out=y[:], in0=y[:], in1=mn[:], op=Alu.add)

                if l < 3:
                    y4b = ep.tile([128, 256], b16, tag="y4b")
                    nc.vector.tensor_scalar_add(out=y4b[:], in0=y[:], scalar1=-1.0)
                    dst_ab = 'A' if l == 1 else 'B'
                    for kk in range(2):
                        tp = pp_ot.tile([128, 128], b16, space="PSUM", tag="ot")
                        nc.tensor.transpose(out=tp[:], in_=y4b[:, kk * 128:(kk + 1) * 128],
                                            identity=identb[:])
                        nc.vector.tensor_copy(out=yT[(dst_ab, kk)][:, b * 128:(b + 1) * 128],
                                              in_=tp[:])
                else:
                    yp = ep.tile([128, 64], f32, tag="yp")
                    nc.vector.tensor_scalar_add(out=yp[:], in0=y[:], scalar1=-1.0)
                    og = ep.tile([128, 256], f32, tag="og")
                    nc.vector.tensor_tensor(out=og[:],
                                            in0=bg_s.to_broadcast([128, 256]),
                                            in1=iot[:], op=Alu.is_equal)
                    for gg in range(2):
                        pg = pp_pg.tile([128, 64], f32, space="PSUM", tag="ot", name="pg")
                        nc.tensor.matmul(out=pg[:], lhsT=og[:, gg * 128:(gg + 1) * 128],
                                         rhs=yp[:], start=True, stop=True)
                        nc.vector.tensor_tensor(out=pacc[gg][:], in0=pacc[gg][:],
                                                in1=pg[:], op=Alu.add)
            if l == 3:
                for gg in range(2):
                    nc.sync.dma_start(out=pout.ap()[gg * 128:(gg + 1) * 128, :],
                                      in_=pacc[gg][:])

        def do_ag(l):
            if 'ag' in ablate:
                return
            li = l - 1
            nc.gpsimd.collective_compute(
                "AllGather", Alu.bypass,
                replica_groups=[list(range(N_CORES))],
                ins=[ag_in[li].ap().opt()], outs=[table[li].ap().opt()])

        # ---- layer 1 ----
        with ExitStack() as es1:
            es_l1 = (es1.enter_context(tc.tile_pool(name="w1", bufs=1)),
                     es1.enter_context(tc.tile_pool(name="xw", bufs=2)))
            mm_phase(1)
        gp = es.enter_context(tc.tile_pool(name="G", bufs=KNOBS["G"]))
        op = es.enter_context(tc.tile_pool(name="onehot", bufs=KNOBS["oh"]))
        ohp = es.enter_context(tc.tile_pool(name="ohT", bufs=KNOBS["oh"]))
        gsp = es.enter_context(tc.tile_pool(name="Gs", bufs=KNOBS["gs"]))
        ep = es.enter_context(tc.tile_pool(name="edge", bufs=KNOBS["ep"]))
        pp_ot = es.enter_context(tc.tile_pool(name="pot", bufs=KNOBS["pot"], space="PSUM"))
        pp_wps = es.enter_context(tc.tile_pool(name="pwps", bufs=KNOBS["wps"], space="PSUM"))
        pp_out = es.enter_context(tc.tile_pool(name="po", bufs=KNOBS["po"], space="PSUM"))
        pp_pg = pp_ot
        pools = (gp, op, ohp, gsp, ep, pp_ot, pp_wps, pp_out, pp_pg)
        w23 = {}
        w23p = es.enter_context(tc.tile_pool(name="w23", bufs=1))
        for l, Wt in ((2, W2e), (3, W3e)):
            w23[l] = []
            for kk in range(2):
                w = w23p.tile([128, 264], b16, tag=f"w{l}_{kk}")
                nc.sync.dma_start(out=w[:], in_=Wt[kk * 128:(kk + 1) * 128, :])
                w23[l].append(w)

        do_ag(1)
        edge_phase(1, pools)
        mm_phase(2)
        do_ag(2)
        edge_phase(2, pools)
        mm_phase(3)
        do_ag(3)
        edge_phase(3, pools)

    nc.compile()
    return nc


# ----------------------------------------------------------------------------
# entry point
# ----------------------------------------------------------------------------

def kernel(**inputs):
    from concourse.bass_utils import run_bass_kernel_spmd

    in_maps, NCH, g_starts, counts = _preprocess(inputs)
    if NCH not in _PROG_CACHE:
        _PROG_CACHE[NCH] = _build_program(NCH)
    nc = _PROG_CACHE[NCH]

    res = run_bass_kernel_spmd(nc, in_maps, core_ids=list(range(N_CORES)))

    full = np.zeros((G_GRAPHS + SPAN, 64), np.float64)
    for k in range(N_CORES):
        full[g_starts[k]:g_starts[k] + SPAN] += res.results[k]['pout'].astype(np.float64)
    out = full[:G_GRAPHS] / np.maximum(counts, 1.0)[:, None]
    return out.astype(np.float32)
